# revision 9
# baseline (speedup 1.0000x reference)
"""BDH dense-transformer Bass kernel for 8 trn2 NeuronCores.

Sharding: tensor-parallel over the 12 heads. Cores are paired; pair p owns
heads {3p, 3p+1, 3p+2}. The even core of the pair runs head 3p fully, the odd
core runs head 3p+1 fully, and head 3p+2 ("S") is query-split between the two
(even core: 256-row q-blocks {0,3}, odd: {1,2} - balanced causal work). The
S-head encoder projection is computed fully on both cores (cheap) so no score
exchange is needed. The per-layer decoder contraction output [E,T] is
AllReduce'd across all 8 cores. lm_head is vocab-split 8 ways.

The SPMD program is identical on all cores: per-core differences come from
input data (weight slices, causal masks for the S strips) and from dynamic
q-offsets derived from partition_id parity.

Layouts (partition dim first):
  xT   [E=768, T=1024]   (6 tiles [128,1024])   - "transposed" x
  x    [T=1024, E=768]   (8 tiles [128,768])    - natural x (V for attention)
  xsT  [N=2048, T=1024]  per head, in DRAM scratch as [16,128,1024]
  scores^T s[k,q] tiles [128k, 256q] (pre-mask scores are symmetric, Q=K)
  softmax along the k (partition) axis: exp on ACT, sums via ones-matmul on PE
  y^T  [E, q256]; LN over partitions via ones-matmul stats
"""

import os
import numpy as np

import concourse.bass as bass
import concourse.tile as tile
from concourse import bacc, mybir
from concourse.bass_utils import run_bass_kernel_spmd

F32 = mybir.dt.float32
AF = mybir.ActivationFunctionType
ALU = mybir.AluOpType

B, T, E, NH, N, VOCAB = 1, 1024, 768, 12, 2048, 32000
NL = 6
LN_EPS = 1e-5
SCALE = 1.0 / float(np.sqrt(N))
NCORES = 8
VS = VOCAB // NCORES  # 4000 vocab slice per core

ET = E // 128   # 6  e-tiles
TT = T // 128   # 8  t-tiles (128)
QW = 256        # q tile width
QT = T // QW    # 4  q-tiles (256)
NT = N // 128   # 16 n-tiles

USE_F32R = os.environ.get("BDH_F32R", "1") == "1"
MMF = mybir.dt.float32r if USE_F32R else mybir.dt.float32

# S-head virtual q-tiles: v=0 -> ki range 4, v=1 -> ki range 8
S_KI = [4, 8]


def _mm(ap):
    return ap


def _host_masks_S(parity: int) -> np.ndarray:
    """Causal masks for the S head strips of this core: [12, 128, 256] f32.

    v=0 entries 0..3 (ki=0..3), v=1 entries 4..11 (ki=0..7).
    valid(k_global <= q_global).
    """
    qphys = {0: [0, 3], 1: [1, 2]}[parity]
    out = np.zeros((sum(S_KI), 128, QW), np.float32)
    slot = 0
    for v in range(2):
        qp = qphys[v]
        for ki in range(S_KI[v]):
            k = 128 * ki + np.arange(128)[:, None]
            q = QW * qp + np.arange(QW)[None, :]
            out[slot] = (k <= q).astype(np.float32)
            slot += 1
    return out


def _host_masks_F() -> np.ndarray:
    """Static diagonal masks for the full head: [2, 128, 256].

    For q-tile qj the two diagonal k-tiles are ki=2qj+d, d in {0,1}:
    valid iff 128*d + p <= f.
    """
    out = np.zeros((2, 128, QW), np.float32)
    for d in range(2):
        k = 128 * d + np.arange(128)[:, None]
        q = np.arange(QW)[None, :]
        out[d] = (k <= q).astype(np.float32)
    return out


def build_program(n_layer: int = NL, with_lm: bool = True):
    nc = bacc.Bacc(
        "TRN2",
        target_bir_lowering=False,
        debug=False,
        num_devices=NCORES,
        name="bdh",
    )

    # ---- kernel I/O ----
    def din(name, shape, dt=F32):
        return nc.dram_tensor(name, list(shape), dt, kind="ExternalInput")

    xe_d = din("xe", (T, E), MMF)
    encF_d = din("encF", (E, N), MMF)
    encS_d = din("encS", (E, N), MMF)
    encvF_d = din("encvF", (E, N), MMF)
    encvS_d = din("encvS", (E, N), MMF)
    decF_d = din("decF", (N, E), MMF)
    decS_d = din("decS", (N, E), MMF)
    lmT_d = din("lmT", (E, VS), MMF)
    lnw_d = din("lnw", (E,))
    lnb_d = din("lnb", (E,))
    maskS_d = din("maskS", (sum(S_KI), 128, QW))
    out_d = nc.dram_tensor("out", [T, VS], F32, kind="ExternalOutput")

    # ---- device constants ----
    ident_np = np.eye(128, dtype=np.float32)
    ones_np = np.ones((128, 1), np.float32)
    ident_d = nc.inline_tensor(ident_np, name="ident128")
    ones_d = nc.inline_tensor(ones_np, name="ones128")
    maskF_d = nc.inline_tensor(_host_masks_F(), name="maskF")

    # ---- DRAM scratch ----
    xsT_dram = [
        nc.dram_tensor(f"xsT_{h}", [NT, 128, T], MMF, kind="Internal")
        for h in range(2)  # 0=F, 1=S
    ]
    cc_in = nc.dram_tensor("cc_in", [E, T], F32, kind="Internal")
    cc_out = nc.dram_tensor("cc_out", [E, T], F32, kind="Internal", addr_space="Shared")

    enc_d = [encF_d, encS_d]
    encv_d = [encvF_d, encvS_d]
    dec_d = [decF_d, decS_d]

    with tile.TileContext(nc) as tc:
        # partition-id parity -> dynamic q offsets for the S head
        pid = nc.partition_id()
        parity = nc.snap(pid % 2, min_val=0, max_val=1)
        qs_dyn = [parity * QW, (3 * QW) - parity * QW]  # v=0, v=1

        import contextlib

        ctx = contextlib.ExitStack()
        with ctx:
            # ---------------- pools ----------------
            pp = ctx.enter_context(tc.tile_pool(name="persist", bufs=1))
            p_xq = ctx.enter_context(tc.tile_pool(name="xq", bufs=1))
            p_xk = ctx.enter_context(tc.tile_pool(name="xk", bufs=1))
            p_exp = ctx.enter_context(tc.tile_pool(name="exp", bufs=10))
            p_wslab = ctx.enter_context(tc.tile_pool(name="wslab", bufs=2))
            p_small = ctx.enter_context(tc.tile_pool(name="small", bufs=2))
            p_ln = ctx.enter_context(tc.tile_pool(name="lnscratch", bufs=1))
            p_yt = ctx.enter_context(tc.tile_pool(name="yt", bufs=6))
            p_yln = ctx.enter_context(tc.tile_pool(name="yln", bufs=6))
            p_stage = ctx.enter_context(tc.tile_pool(name="stage", bufs=2))
            ps_main = ctx.enter_context(tc.tile_pool(name="psmain", bufs=4, space="PSUM"))
            ps_stat = ctx.enter_context(tc.tile_pool(name="psstat", bufs=2, space="PSUM"))

            # ---------------- persistent SBUF ----------------
            ident = pp.tile([128, 128], F32, tag="ident")
            nc.sync.dma_start(ident[:, :], ident_d[:, :])
            ones = pp.tile([128, 1], F32, tag="ones")
            nc.sync.dma_start(ones[:, :], ones_d[:, :])
            maskF = pp.tile([128, 2, QW], F32, tag="maskF")
            nc.sync.dma_start(maskF[:, :, :], maskF_d.ap().rearrange("d p q -> p d q"))
            maskS = pp.tile([128, sum(S_KI), QW], F32, tag="maskS")
            nc.sync.dma_start(maskS[:, :, :], maskS_d.ap().rearrange("s p q -> p s q"))

            lnw = pp.tile([128, ET], F32, tag="lnw")
            nc.sync.dma_start(
                lnw[:, :], lnw_d.ap().rearrange("(c p) -> p c", p=128)
            )
            lnb = pp.tile([128, ET], F32, tag="lnb")
            nc.sync.dma_start(
                lnb[:, :], lnb_d.ap().rearrange("(c p) -> p c", p=128)
            )

            eps_t = pp.tile([1, 1], F32, tag="eps")
            nc.vector.memset(eps_t[:, :], float(LN_EPS))

            xT = [pp.tile([128, T], MMF, tag=f"xT{e}", name=f"xT{e}") for e in range(ET)]
            xn = [pp.tile([128, E], MMF, tag=f"xn{t}", name=f"xn{t}") for t in range(TT)]
            ymlp = [pp.tile([128, T], F32, tag=f"ym{e}", name=f"ym{e}") for e in range(ET)]

            # ---------------- helpers ----------------
            def sum_partitions(dst_ps, src_ap):
                """dst_ps [1,w] += ones^T @ src  (sum over 128 partitions)."""
                nc.tensor.matmul(dst_ps, ones[:, :], src_ap, start=True, stop=True)

            def transpose128(dst_sb, src_ap):
                """dst[128,128] = src.T via PE transpose + ACT copy."""
                ps = ps_main.tile([128, 512], F32, tag="ps")
                nc.tensor.transpose(ps[:, 0:128], src_ap.bitcast(F32), ident[:, :])
                nc.scalar.copy(dst_sb, ps[:, 0:128])

            def ln_partition(in_aps, out_aps, width):
                """LayerNorm over the partition axis (768 rows as 6x128 tiles).

                in_aps/out_aps: lists of 6 APs [128, width]. width <= 512.
                """
                assert width <= 512
                # sum x
                a = p_ln.tile([128, width], F32, tag="ln_a")
                nc.vector.tensor_add(a[:, :], in_aps[0], in_aps[1])
                for e in range(2, ET):
                    nc.vector.tensor_add(a[:, :], a[:, :], in_aps[e])
                s1 = ps_stat.tile([1, 512], F32, tag="st")
                sum_partitions(s1[0:1, 0:width], a[:, :])
                # sum x^2
                b = p_ln.tile([128, width], F32, tag="ln_b")
                sq = p_ln.tile([128, width], F32, tag="ln_sq")
                nc.scalar.square(b[:, :], in_aps[0])
                for e in range(1, ET):
                    nc.scalar.square(sq[:, :], in_aps[e])
                    nc.vector.tensor_add(b[:, :], b[:, :], sq[:, :])
                s2 = ps_stat.tile([1, 512], F32, tag="st")
                sum_partitions(s2[0:1, 0:width], b[:, :])
                # stats
                m = p_small.tile([1, 512], F32, tag="sm_m")
                nc.vector.tensor_scalar(
                    m[0:1, 0:width], s1[0:1, 0:width], 1.0 / E, None, ALU.mult
                )
                var = p_small.tile([1, 512], F32, tag="sm_v")
                nc.vector.tensor_scalar(
                    var[0:1, 0:width], s2[0:1, 0:width], 1.0 / E, None, ALU.mult
                )
                msq = p_small.tile([1, 512], F32, tag="sm_t")
                nc.vector.tensor_mul(msq[0:1, 0:width], m[0:1, 0:width], m[0:1, 0:width])
                nc.vector.tensor_sub(var[0:1, 0:width], var[0:1, 0:width], msq[0:1, 0:width])
                sd = p_small.tile([1, 512], F32, tag="sm_t")
                nc.scalar.activation(
                    sd[0:1, 0:width], var[0:1, 0:width], AF.Sqrt, bias=eps_t[0:1, 0:1]
                )
                r = p_small.tile([1, 512], F32, tag="sm_r")
                nc.vector.reciprocal(r[0:1, 0:width], sd[0:1, 0:width])
                v = p_small.tile([1, 512], F32, tag="sm_t")
                nc.vector.tensor_mul(v[0:1, 0:width], m[0:1, 0:width], r[0:1, 0:width])
                nc.vector.tensor_scalar(
                    v[0:1, 0:width], v[0:1, 0:width], -1.0, None, ALU.mult
                )
                rb = p_ln.tile([128, width], F32, tag="ln_rb")
                nc.gpsimd.partition_broadcast(rb[:, :], r[0:1, 0:width])
                vb = p_ln.tile([128, width], F32, tag="ln_vb")
                nc.gpsimd.partition_broadcast(vb[:, :], v[0:1, 0:width])
                # apply: out = (x*rb + vb)*w + b
                t1 = p_ln.tile([128, width], F32, tag="ln_t1")
                for e in range(ET):
                    nc.vector.tensor_mul(t1[:, :], in_aps[e], rb[:, :])
                    nc.vector.tensor_add(t1[:, :], t1[:, :], vb[:, :])
                    nc.vector.tensor_scalar(
                        out_aps[e],
                        t1[:, :],
                        lnw[:, e : e + 1],
                        lnb[:, e : e + 1],
                        ALU.mult,
                        ALU.add,
                    )

            # ---------------- embedding: xe -> xT (ln) and x ----------------
            for t in range(TT):
                nc.sync.dma_start(xn[t][:, :], xe_d[128 * t : 128 * (t + 1), :])
            # transpose xe into xT (raw), then LN in place, then transpose back
            for e in range(ET):
                for t in range(TT):
                    transpose128(
                        xT[e][:, 128 * t : 128 * (t + 1)],
                        xn[t][:, 128 * e : 128 * (e + 1)],
                    )
            for half in range(2):
                sl = slice(512 * half, 512 * (half + 1))
                ln_partition([xT[e][:, sl] for e in range(ET)],
                             [xT[e][:, sl] for e in range(ET)], 512)
            for t in range(TT):
                for e in range(ET):
                    transpose128(
                        xn[t][:, 128 * e : 128 * (e + 1)],
                        xT[e][:, 128 * t : 128 * (t + 1)],
                    )

            # ---------------- layers ----------------
            for layer in range(n_layer):
                # ---- Phase A: encoder projections (both heads, full T) ----
                for h in range(2):
                    for g in range(8):  # n-groups of 2x128
                        slab = p_wslab.tile([128, ET, 256], MMF, tag="wslab")
                        nc.sync.dma_start(
                            slab[:, :, :],
                            enc_d[h]
                            .ap()
                            .rearrange("(c p) n -> p c n", p=128)[
                                :, :, 256 * g : 256 * (g + 1)
                            ],
                        )
                        for j in range(2):
                            ncx = 2 * g + j
                            row = p_stage.tile([128, T], MMF, tag="xsrow")
                            for tt in range(2):
                                ps = ps_main.tile([128, 512], F32, tag="ps")
                                for e in range(ET):
                                    nc.tensor.matmul(
                                        ps[:, :],
                                        _mm(slab[:, e, 128 * j : 128 * (j + 1)]),
                                        _mm(xT[e][:, 512 * tt : 512 * (tt + 1)]),
                                        start=(e == 0),
                                        stop=(e == ET - 1),
                                    )
                                nc.scalar.activation(
                                    row[:, 512 * tt : 512 * (tt + 1)], ps[:, :], AF.Relu
                                )
                            nc.sync.dma_start(xsT_dram[h][ncx, :, :], row[:, :])

                # ---- Phase B: per-head, per-q-tile attention + mlp ----
                for h in range(2):
                    q_iters = range(QT) if h == 0 else range(2)
                    for qi in q_iters:
                        if h == 0:
                            qs = QW * qi
                            kis = 2 * qi + 2
                        else:
                            qs = qs_dyn[qi]
                            kis = S_KI[qi]

                        # q-slice of xsT (scores rhs + xy operand)
                        xq = p_xq.tile([128, NT, QW], MMF, tag="xq")
                        nc.sync.dma_start(
                            xq[:, :, :],
                            xsT_dram[h]
                            .ap()
                            .rearrange("n p t -> p n t")[:, :, bass.ds(qs, QW)]
                            if h == 1
                            else xsT_dram[h]
                            .ap()
                            .rearrange("n p t -> p n t")[:, :, qs : qs + QW],
                        )

                        # scores + exp
                        exps = []
                        for ki in range(kis):
                            xk = p_xk.tile([128, NT, 128], MMF, tag="xk")
                            nc.sync.dma_start(
                                xk[:, :, :],
                                xsT_dram[h]
                                .ap()
                                .rearrange("n p t -> p n t")[
                                    :, :, 128 * ki : 128 * (ki + 1)
                                ],
                            )
                            ps = ps_main.tile([128, 512], F32, tag="ps")
                            for ncx in range(NT):
                                nc.tensor.matmul(
                                    ps[:, 0:QW],
                                    _mm(xk[:, ncx, :]),
                                    _mm(xq[:, ncx, :]),
                                    start=(ncx == 0),
                                    stop=(ncx == NT - 1),
                                )
                            et = p_exp.tile([128, QW], MMF, tag="exp")
                            nc.scalar.activation(
                                et[:, :], ps[:, 0:QW], AF.Exp, scale=float(SCALE)
                            )
                            if h == 0:
                                d = ki - 2 * qi
                                if d >= 0:
                                    nc.vector.tensor_mul(
                                        et[:, :], et[:, :], maskF[:, d, :]
                                    )
                            else:
                                slot = (0 if qi == 0 else S_KI[0]) + ki
                                nc.vector.tensor_mul(
                                    et[:, :], et[:, :], maskS[:, slot, :]
                                )
                            exps.append(et)

                        # Z = sum_k exp, then 1/Z broadcast
                        acc = p_stage.tile([128, QW], F32, tag="zacc")
                        if kis == 1:
                            nc.vector.tensor_copy(acc[:, :], exps[0][:, :])
                        else:
                            nc.vector.tensor_add(acc[:, :], exps[0][:, :], exps[1][:, :])
                            for ki in range(2, kis):
                                nc.vector.tensor_add(acc[:, :], acc[:, :], exps[ki][:, :])
                        zp = ps_stat.tile([1, 512], F32, tag="st")
                        sum_partitions(zp[0:1, 0:QW], acc[:, :])
                        zr = p_small.tile([1, 512], F32, tag="sm_z")
                        nc.vector.reciprocal(zr[0:1, 0:QW], zp[0:1, 0:QW])
                        rb = p_stage.tile([128, QW], F32, tag="zrb")
                        nc.gpsimd.partition_broadcast(rb[:, :], zr[0:1, 0:QW])

                        # yT = (expS^T-weighted V) * (1/Z), V = x natural
                        yts = []
                        for e in range(ET):
                            ps = ps_main.tile([128, 512], F32, tag="ps")
                            for ki in range(kis):
                                nc.tensor.matmul(
                                    ps[:, 0:QW],
                                    _mm(xn[ki][:, 128 * e : 128 * (e + 1)]),
                                    _mm(exps[ki][:, :]),
                                    start=(ki == 0),
                                    stop=(ki == kis - 1),
                                )
                            yt = p_yt.tile([128, QW], F32, tag="yt")
                            nc.vector.tensor_mul(yt[:, :], ps[:, 0:QW], rb[:, :])
                            yts.append(yt)

                        # ln(y)
                        ylns = [p_yln.tile([128, QW], MMF, tag="yln", name=f"yln{_}") for _ in range(ET)]
                        ln_partition([y[:, :] for y in yts], [y[:, :] for y in ylns], QW)

                        # encoder_v + xy (in place into xq)
                        for g in range(8):
                            slab = p_wslab.tile([128, ET, 256], MMF, tag="wslab")
                            nc.sync.dma_start(
                                slab[:, :, :],
                                encv_d[h]
                                .ap()
                                .rearrange("(c p) n -> p c n", p=128)[
                                    :, :, 256 * g : 256 * (g + 1)
                                ],
                            )
                            for j in range(2):
                                ncx = 2 * g + j
                                ps = ps_main.tile([128, 512], F32, tag="ps")
                                for e in range(ET):
                                    nc.tensor.matmul(
                                        ps[:, 0:QW],
                                        _mm(slab[:, e, 128 * j : 128 * (j + 1)]),
                                        _mm(ylns[e][:, :]),
                                        start=(e == 0),
                                        stop=(e == ET - 1),
                                    )
                                ys = p_stage.tile([128, QW], F32, tag="ys")
                                nc.scalar.activation(ys[:, :], ps[:, 0:QW], AF.Relu)
                                nc.vector.tensor_mul(
                                    xq[:, ncx, :], ys[:, :], xq[:, ncx, :]
                                )

                        # decoder: ymlp[e, qs:qs+QW] (+)= sum_n dec[n,e]^T xy[n,q]
                        for e in range(ET):
                            slab = p_wslab.tile([128, NT, 128], MMF, tag="wslab")
                            nc.sync.dma_start(
                                slab[:, :, :],
                                dec_d[h]
                                .ap()
                                .rearrange("(n p) c -> p n c", p=128)[
                                    :, :, 128 * e : 128 * (e + 1)
                                ],
                            )
                            ps = ps_main.tile([128, 512], F32, tag="ps")
                            for ncx in range(NT):
                                nc.tensor.matmul(
                                    ps[:, 0:QW],
                                    _mm(slab[:, ncx, :]),
                                    _mm(xq[:, ncx, :]),
                                    start=(ncx == 0),
                                    stop=(ncx == NT - 1),
                                )
                            if h == 0:
                                nc.scalar.copy(ymlp[e][:, qs : qs + QW], ps[:, 0:QW])
                            else:
                                dst = ymlp[e][:, bass.ds(qs, QW)]
                                nc.vector.tensor_add(dst, dst, ps[:, 0:QW])

                # ---- Phase C: AllReduce + LN chain + transpose ----
                for e in range(ET):
                    nc.sync.dma_start(
                        cc_in[128 * e : 128 * (e + 1), :], ymlp[e][:, :]
                    )
                nc.gpsimd.collective_compute(
                    "AllReduce",
                    ALU.add,
                    replica_groups=[list(range(NCORES))],
                    ins=[cc_in.ap().opt()],
                    outs=[cc_out.ap().opt()],
                )
                ym = ymlp
                for e in range(ET):
                    nc.sync.dma_start(ym[e][:, :], cc_out[128 * e : 128 * (e + 1), :])
                for half in range(2):
                    sl = slice(512 * half, 512 * (half + 1))
                    # ln(y_mlp)
                    ln_partition(
                        [ym[e][:, sl] for e in range(ET)],
                        [ym[e][:, sl] for e in range(ET)],
                        512,
                    )
                    # x + ln(y_mlp)
                    for e in range(ET):
                        nc.vector.tensor_add(ym[e][:, sl], ym[e][:, sl], xT[e][:, sl])
                    # ln(...) -> new xT
                    ln_partition(
                        [ym[e][:, sl] for e in range(ET)],
                        [xT[e][:, sl] for e in range(ET)],
                        512,
                    )
                # transpose xT -> x natural
                for t in range(TT):
                    for e in range(ET):
                        transpose128(
                            xn[t][:, 128 * e : 128 * (e + 1)],
                            xT[e][:, 128 * t : 128 * (t + 1)],
                        )

            # ---------------- lm_head ----------------
            if with_lm:
                nvt = (VS + 255) // 256  # 16 v-tiles of <=256
                for vt in range(nvt):
                    w = min(256, VS - 256 * vt)
                    slab = p_wslab.tile([128, ET, 256], MMF, tag="wslab")
                    nc.sync.dma_start(
                        slab[:, :, 0:w],
                        lmT_d.ap().rearrange("(c p) v -> p c v", p=128)[
                            :, :, 256 * vt : 256 * vt + w
                        ],
                    )
                    for t in range(TT):
                        ps = ps_main.tile([128, 512], F32, tag="ps")
                        for e in range(ET):
                            nc.tensor.matmul(
                                ps[:, 0:w],
                                _mm(xT[e][:, 128 * t : 128 * (t + 1)]),
                                _mm(slab[:, e, 0:w]),
                                start=(e == 0),
                                stop=(e == ET - 1),
                            )
                        ot = p_stage.tile([128, 256], F32, tag="lmout")
                        nc.scalar.copy(ot[:, 0:w], ps[:, 0:w])
                        nc.sync.dma_start(
                            out_d[128 * t : 128 * (t + 1), 256 * vt : 256 * vt + w],
                            ot[:, 0:w],
                        )

    nc.compile()
    return nc


_CACHED_NC = None


def _get_nc():
    global _CACHED_NC
    if _CACHED_NC is None:
        _CACHED_NC = build_program()
    return _CACHED_NC


def make_in_maps(idx, embed, encoder, encoder_v, decoder, lm_head, ln_w, ln_b):
    idx = np.asarray(idx)
    embed = np.asarray(embed, dtype=np.float32)
    encoder = np.asarray(encoder, dtype=np.float32)
    encoder_v = np.asarray(encoder_v, dtype=np.float32)
    decoder = np.asarray(decoder, dtype=np.float32)
    lm_head = np.asarray(lm_head, dtype=np.float32)
    ln_w = np.asarray(ln_w, dtype=np.float32)
    ln_b = np.asarray(ln_b, dtype=np.float32)
    assert idx.shape == (B, T)

    xe = embed[idx.astype(np.int64).reshape(-1)]  # [T, E] host gather
    dec3 = decoder.reshape(NH, N, E)

    in_maps = []
    for c in range(NCORES):
        p = c // 2
        par = c % 2
        hF = 3 * p + par
        hS = 3 * p + 2
        in_maps.append(
            {
                "xe": np.ascontiguousarray(xe),
                "encF": np.ascontiguousarray(encoder[hF]),
                "encS": np.ascontiguousarray(encoder[hS]),
                "encvF": np.ascontiguousarray(encoder_v[hF]),
                "encvS": np.ascontiguousarray(encoder_v[hS]),
                "decF": np.ascontiguousarray(dec3[hF]),
                "decS": np.ascontiguousarray(dec3[hS]),
                "lmT": np.ascontiguousarray(lm_head[VS * c : VS * (c + 1), :].T),
                "lnw": np.ascontiguousarray(ln_w),
                "lnb": np.ascontiguousarray(ln_b),
                "maskS": _host_masks_S(par),
            }
        )
    return in_maps


def kernel(idx, embed, encoder, encoder_v, decoder, lm_head, ln_w, ln_b, n_layer):
    assert int(n_layer) == NL
    in_maps = make_in_maps(
        idx, embed, encoder, encoder_v, decoder, lm_head, ln_w, ln_b
    )
    nc = _get_nc()
    res = run_bass_kernel_spmd(nc, in_maps, core_ids=list(range(NCORES)))
    logits = np.concatenate(
        [res.results[c]["out"] for c in range(NCORES)], axis=1
    )  # [T, VOCAB]
    return logits.reshape(B, T, VOCAB).astype(np.float32)


if __name__ == "__main__":
    nc = build_program(n_layer=1, with_lm=False)
    print("built ok")


# revision 10
# speedup vs baseline: 1.0229x; 1.0229x over previous
"""BDH dense-transformer Bass kernel for 8 trn2 NeuronCores.

Sharding: tensor-parallel over the 12 heads. Cores are paired; pair p owns
heads {3p, 3p+1, 3p+2}. The even core of the pair runs head 3p fully, the odd
core runs head 3p+1 fully, and head 3p+2 ("S") is query-split between the two
(even core: 256-row q-blocks {0,3}, odd: {1,2} - balanced causal work). The
S-head encoder projection is computed fully on both cores (cheap) so no score
exchange is needed. The per-layer decoder contraction output [E,T] is
AllReduce'd across all 8 cores. lm_head is vocab-split 8 ways.

The SPMD program is identical on all cores: per-core differences come from
input data (weight slices, causal masks for the S strips) and from dynamic
q-offsets derived from partition_id parity.

Layouts (partition dim first):
  xT   [E=768, T=1024]   (6 tiles [128,1024])   - "transposed" x
  x    [T=1024, E=768]   (8 tiles [128,768])    - natural x (V for attention)
  xsT  [N=2048, T=1024]  per head, in DRAM scratch as [16,128,1024]
  scores^T s[k,q] tiles [128k, 256q] (pre-mask scores are symmetric, Q=K)
  softmax along the k (partition) axis: exp on ACT, sums via ones-matmul on PE
  y^T  [E, q256]; LN over partitions via ones-matmul stats
"""

import os
import numpy as np

import concourse.bass as bass
import concourse.tile as tile
from concourse import bacc, mybir
from concourse.bass_utils import run_bass_kernel_spmd

F32 = mybir.dt.float32
AF = mybir.ActivationFunctionType
ALU = mybir.AluOpType

B, T, E, NH, N, VOCAB = 1, 1024, 768, 12, 2048, 32000
NL = 6
LN_EPS = 1e-5
SCALE = 1.0 / float(np.sqrt(N))
NCORES = 8
VS = VOCAB // NCORES  # 4000 vocab slice per core

ET = E // 128   # 6  e-tiles
TT = T // 128   # 8  t-tiles (128)
QW = 256        # q tile width
QT = T // QW    # 4  q-tiles (256)
NT = N // 128   # 16 n-tiles

USE_F32R = os.environ.get("BDH_F32R", "1") == "1"
MMF = mybir.dt.float32r if USE_F32R else mybir.dt.float32

# S-head virtual q-tiles: v=0 -> ki range 4, v=1 -> ki range 8
S_KI = [4, 8]


def _mm(ap):
    return ap


def _host_masks_S(parity: int) -> np.ndarray:
    """Causal masks for the S head strips of this core: [12, 128, 256] f32.

    v=0 entries 0..3 (ki=0..3), v=1 entries 4..11 (ki=0..7).
    valid(k_global <= q_global).
    """
    qphys = {0: [0, 3], 1: [1, 2]}[parity]
    out = np.zeros((sum(S_KI), 128, QW), np.float32)
    slot = 0
    for v in range(2):
        qp = qphys[v]
        for ki in range(S_KI[v]):
            k = 128 * ki + np.arange(128)[:, None]
            q = QW * qp + np.arange(QW)[None, :]
            out[slot] = (k <= q).astype(np.float32)
            slot += 1
    return out


def _host_masks_F() -> np.ndarray:
    """Static diagonal masks for the full head: [2, 128, 256].

    For q-tile qj the two diagonal k-tiles are ki=2qj+d, d in {0,1}:
    valid iff 128*d + p <= f.
    """
    out = np.zeros((2, 128, QW), np.float32)
    for d in range(2):
        k = 128 * d + np.arange(128)[:, None]
        q = np.arange(QW)[None, :]
        out[d] = (k <= q).astype(np.float32)
    return out


def build_program(n_layer: int = NL, with_lm: bool = True, use_cc: bool = True):
    nc = bacc.Bacc(
        "TRN2",
        target_bir_lowering=False,
        debug=False,
        num_devices=NCORES,
        name="bdh",
    )

    # ---- kernel I/O ----
    def din(name, shape, dt=F32):
        return nc.dram_tensor(name, list(shape), dt, kind="ExternalInput")

    xe_d = din("xe", (T, E), MMF)
    encF_d = din("encF", (E, N), MMF)
    encS_d = din("encS", (E, N), MMF)
    encvF_d = din("encvF", (E, N), MMF)
    encvS_d = din("encvS", (E, N), MMF)
    decF_d = din("decF", (N, E), MMF)
    decS_d = din("decS", (N, E), MMF)
    lmT_d = din("lmT", (E, VS), MMF)
    lnw_d = din("lnw", (E,))
    lnb_d = din("lnb", (E,))
    maskS_d = din("maskS", (sum(S_KI), 128, QW))
    out_d = nc.dram_tensor("out", [T, VS], F32, kind="ExternalOutput")

    # ---- device constants ----
    ident_np = np.eye(128, dtype=np.float32)
    ones_np = np.ones((128, 1), np.float32)
    ident_d = nc.inline_tensor(ident_np, name="ident128")
    ones_d = nc.inline_tensor(ones_np, name="ones128")
    maskF_d = nc.inline_tensor(_host_masks_F(), name="maskF")

    # ---- DRAM scratch ----
    xsT_dram = [
        nc.dram_tensor(f"xsT_{h}", [NT, 128, T], MMF, kind="Internal")
        for h in range(2)  # 0=F, 1=S
    ]
    cc_in = nc.dram_tensor("cc_in", [E, T], F32, kind="Internal")
    cc_out = nc.dram_tensor("cc_out", [E, T], F32, kind="Internal", addr_space="Shared")

    enc_d = [encF_d, encS_d]
    encv_d = [encvF_d, encvS_d]
    dec_d = [decF_d, decS_d]

    with tile.TileContext(nc) as tc:
        # partition-id parity -> dynamic q offsets for the S head
        pid = nc.partition_id()
        parity = nc.snap(pid % 2, min_val=0, max_val=1)
        qs_dyn = [parity * QW, (3 * QW) - parity * QW]  # v=0, v=1

        import contextlib

        ctx = contextlib.ExitStack()
        with ctx:
            # ---------------- pools ----------------
            pp = ctx.enter_context(tc.tile_pool(name="persist", bufs=1))
            p_xq = ctx.enter_context(tc.tile_pool(name="xq", bufs=1))
            p_xk = ctx.enter_context(tc.tile_pool(name="xk", bufs=1))
            p_exp = ctx.enter_context(tc.tile_pool(name="exp", bufs=10))
            p_wslab = ctx.enter_context(tc.tile_pool(name="wslab", bufs=2))
            p_small = ctx.enter_context(tc.tile_pool(name="small", bufs=2))
            p_ln = ctx.enter_context(tc.tile_pool(name="lnscratch", bufs=1))
            p_yt = ctx.enter_context(tc.tile_pool(name="yt", bufs=6))
            p_yln = ctx.enter_context(tc.tile_pool(name="yln", bufs=6))
            p_stage = ctx.enter_context(tc.tile_pool(name="stage", bufs=2))
            ps_main = ctx.enter_context(tc.tile_pool(name="psmain", bufs=4, space="PSUM"))
            ps_stat = ctx.enter_context(tc.tile_pool(name="psstat", bufs=2, space="PSUM"))

            # ---------------- persistent SBUF ----------------
            ident = pp.tile([128, 128], F32, tag="ident")
            nc.sync.dma_start(ident[:, :], ident_d[:, :])
            ones = pp.tile([128, 1], F32, tag="ones")
            nc.sync.dma_start(ones[:, :], ones_d[:, :])
            maskF = pp.tile([128, 2, QW], F32, tag="maskF")
            nc.sync.dma_start(maskF[:, :, :], maskF_d.ap().rearrange("d p q -> p d q"))
            maskS = pp.tile([128, sum(S_KI), QW], F32, tag="maskS")
            nc.sync.dma_start(maskS[:, :, :], maskS_d.ap().rearrange("s p q -> p s q"))

            lnw = pp.tile([128, ET], F32, tag="lnw")
            nc.sync.dma_start(
                lnw[:, :], lnw_d.ap().rearrange("(c p) -> p c", p=128)
            )
            lnb = pp.tile([128, ET], F32, tag="lnb")
            nc.sync.dma_start(
                lnb[:, :], lnb_d.ap().rearrange("(c p) -> p c", p=128)
            )

            eps_t = pp.tile([1, 1], F32, tag="eps")
            nc.vector.memset(eps_t[:, :], float(LN_EPS))

            xT = [pp.tile([128, T], MMF, tag=f"xT{e}", name=f"xT{e}") for e in range(ET)]
            xn = [pp.tile([128, E], MMF, tag=f"xn{t}", name=f"xn{t}") for t in range(TT)]
            ymlp = [pp.tile([128, T], F32, tag=f"ym{e}", name=f"ym{e}") for e in range(ET)]

            # ---------------- helpers ----------------
            def sum_partitions(dst_ps, src_ap):
                """dst_ps [1,w] += ones^T @ src  (sum over 128 partitions)."""
                nc.tensor.matmul(dst_ps, ones[:, :], src_ap, start=True, stop=True)

            def transpose128(dst_sb, src_ap):
                """dst[128,128] = src.T via PE transpose + ACT copy."""
                ps = ps_main.tile([128, 512], F32, tag="ps")
                nc.tensor.transpose(ps[:, 0:128], src_ap.bitcast(F32), ident[:, :])
                nc.scalar.copy(dst_sb, ps[:, 0:128])

            def ln_partition(in_aps, out_aps, width):
                """LayerNorm over the partition axis (768 rows as 6x128 tiles).

                in_aps/out_aps: lists of 6 APs [128, width]. width <= 512.
                """
                assert width <= 512
                # sum x
                a = p_ln.tile([128, width], F32, tag="ln_a")
                nc.vector.tensor_add(a[:, :], in_aps[0], in_aps[1])
                for e in range(2, ET):
                    nc.vector.tensor_add(a[:, :], a[:, :], in_aps[e])
                s1 = ps_stat.tile([1, 512], F32, tag="st")
                sum_partitions(s1[0:1, 0:width], a[:, :])
                # sum x^2
                b = p_ln.tile([128, width], F32, tag="ln_b")
                sq = p_ln.tile([128, width], F32, tag="ln_sq")
                nc.scalar.square(b[:, :], in_aps[0])
                for e in range(1, ET):
                    nc.scalar.square(sq[:, :], in_aps[e])
                    nc.vector.tensor_add(b[:, :], b[:, :], sq[:, :])
                s2 = ps_stat.tile([1, 512], F32, tag="st")
                sum_partitions(s2[0:1, 0:width], b[:, :])
                # stats
                m = p_small.tile([1, 512], F32, tag="sm_m")
                nc.vector.tensor_scalar(
                    m[0:1, 0:width], s1[0:1, 0:width], 1.0 / E, None, ALU.mult
                )
                var = p_small.tile([1, 512], F32, tag="sm_v")
                nc.vector.tensor_scalar(
                    var[0:1, 0:width], s2[0:1, 0:width], 1.0 / E, None, ALU.mult
                )
                msq = p_small.tile([1, 512], F32, tag="sm_t")
                nc.vector.tensor_mul(msq[0:1, 0:width], m[0:1, 0:width], m[0:1, 0:width])
                nc.vector.tensor_sub(var[0:1, 0:width], var[0:1, 0:width], msq[0:1, 0:width])
                sd = p_small.tile([1, 512], F32, tag="sm_t")
                nc.scalar.activation(
                    sd[0:1, 0:width], var[0:1, 0:width], AF.Sqrt, bias=eps_t[0:1, 0:1]
                )
                r = p_small.tile([1, 512], F32, tag="sm_r")
                nc.vector.reciprocal(r[0:1, 0:width], sd[0:1, 0:width])
                v = p_small.tile([1, 512], F32, tag="sm_t")
                nc.vector.tensor_mul(v[0:1, 0:width], m[0:1, 0:width], r[0:1, 0:width])
                nc.vector.tensor_scalar(
                    v[0:1, 0:width], v[0:1, 0:width], -1.0, None, ALU.mult
                )
                rb = p_ln.tile([128, width], F32, tag="ln_rb")
                nc.gpsimd.partition_broadcast(rb[:, :], r[0:1, 0:width])
                vb = p_ln.tile([128, width], F32, tag="ln_vb")
                nc.gpsimd.partition_broadcast(vb[:, :], v[0:1, 0:width])
                # apply: out = (x*rb + vb)*w + b
                t1 = p_ln.tile([128, width], F32, tag="ln_t1")
                for e in range(ET):
                    nc.vector.tensor_mul(t1[:, :], in_aps[e], rb[:, :])
                    nc.vector.tensor_add(t1[:, :], t1[:, :], vb[:, :])
                    nc.vector.tensor_scalar(
                        out_aps[e],
                        t1[:, :],
                        lnw[:, e : e + 1],
                        lnb[:, e : e + 1],
                        ALU.mult,
                        ALU.add,
                    )

            # ---------------- embedding: xe -> xT (ln) and x ----------------
            for t in range(TT):
                nc.sync.dma_start(xn[t][:, :], xe_d[128 * t : 128 * (t + 1), :])
            # transpose xe into xT (raw), then LN in place, then transpose back
            for e in range(ET):
                for t in range(TT):
                    transpose128(
                        xT[e][:, 128 * t : 128 * (t + 1)],
                        xn[t][:, 128 * e : 128 * (e + 1)],
                    )
            for half in range(2):
                sl = slice(512 * half, 512 * (half + 1))
                ln_partition([xT[e][:, sl] for e in range(ET)],
                             [xT[e][:, sl] for e in range(ET)], 512)
            for t in range(TT):
                for e in range(ET):
                    transpose128(
                        xn[t][:, 128 * e : 128 * (e + 1)],
                        xT[e][:, 128 * t : 128 * (t + 1)],
                    )

            # ---------------- layers ----------------
            for layer in range(n_layer):
                # ---- Phase A: encoder projections (both heads, full T) ----
                for h in range(2):
                    for g in range(8):  # n-groups of 2x128
                        slab = p_wslab.tile([128, ET, 256], MMF, tag="wslab")
                        nc.sync.dma_start(
                            slab[:, :, :],
                            enc_d[h]
                            .ap()
                            .rearrange("(c p) n -> p c n", p=128)[
                                :, :, 256 * g : 256 * (g + 1)
                            ],
                        )
                        for j in range(2):
                            ncx = 2 * g + j
                            row = p_stage.tile([128, T], MMF, tag="xsrow")
                            for tt in range(2):
                                ps = ps_main.tile([128, 512], F32, tag="ps")
                                for e in range(ET):
                                    nc.tensor.matmul(
                                        ps[:, :],
                                        _mm(slab[:, e, 128 * j : 128 * (j + 1)]),
                                        _mm(xT[e][:, 512 * tt : 512 * (tt + 1)]),
                                        start=(e == 0),
                                        stop=(e == ET - 1),
                                    )
                                nc.scalar.activation(
                                    row[:, 512 * tt : 512 * (tt + 1)], ps[:, :], AF.Relu
                                )
                            nc.sync.dma_start(xsT_dram[h][ncx, :, :], row[:, :])

                # ---- Phase B: per-head, per-q-tile attention + mlp ----
                for h in range(2):
                    q_iters = range(QT) if h == 0 else range(2)
                    for qi in q_iters:
                        if h == 0:
                            qs = QW * qi
                            kis = 2 * qi + 2
                        else:
                            qs = qs_dyn[qi]
                            kis = S_KI[qi]

                        # q-slice of xsT (scores rhs + xy operand)
                        xq = p_xq.tile([128, NT, QW], MMF, tag="xq")
                        nc.sync.dma_start(
                            xq[:, :, :],
                            xsT_dram[h]
                            .ap()
                            .rearrange("n p t -> p n t")[:, :, bass.ds(qs, QW)]
                            if h == 1
                            else xsT_dram[h]
                            .ap()
                            .rearrange("n p t -> p n t")[:, :, qs : qs + QW],
                        )

                        # scores + exp
                        exps = []
                        for ki in range(kis):
                            xk = p_xk.tile([128, NT, 128], MMF, tag="xk")
                            nc.sync.dma_start(
                                xk[:, :, :],
                                xsT_dram[h]
                                .ap()
                                .rearrange("n p t -> p n t")[
                                    :, :, 128 * ki : 128 * (ki + 1)
                                ],
                            )
                            ps = ps_main.tile([128, 512], F32, tag="ps")
                            for ncx in range(NT):
                                nc.tensor.matmul(
                                    ps[:, 0:QW],
                                    _mm(xk[:, ncx, :]),
                                    _mm(xq[:, ncx, :]),
                                    start=(ncx == 0),
                                    stop=(ncx == NT - 1),
                                )
                            et = p_exp.tile([128, QW], MMF, tag="exp")
                            nc.scalar.activation(
                                et[:, :], ps[:, 0:QW], AF.Exp, scale=float(SCALE)
                            )
                            if h == 0:
                                d = ki - 2 * qi
                                if d >= 0:
                                    nc.vector.tensor_mul(
                                        et[:, :], et[:, :], maskF[:, d, :]
                                    )
                            else:
                                slot = (0 if qi == 0 else S_KI[0]) + ki
                                nc.vector.tensor_mul(
                                    et[:, :], et[:, :], maskS[:, slot, :]
                                )
                            exps.append(et)

                        # Z = sum_k exp, then 1/Z broadcast
                        acc = p_stage.tile([128, QW], F32, tag="zacc")
                        if kis == 1:
                            nc.vector.tensor_copy(acc[:, :], exps[0][:, :])
                        else:
                            nc.vector.tensor_add(acc[:, :], exps[0][:, :], exps[1][:, :])
                            for ki in range(2, kis):
                                nc.vector.tensor_add(acc[:, :], acc[:, :], exps[ki][:, :])
                        zp = ps_stat.tile([1, 512], F32, tag="st")
                        sum_partitions(zp[0:1, 0:QW], acc[:, :])
                        zr = p_small.tile([1, 512], F32, tag="sm_z")
                        nc.vector.reciprocal(zr[0:1, 0:QW], zp[0:1, 0:QW])
                        rb = p_stage.tile([128, QW], F32, tag="zrb")
                        nc.gpsimd.partition_broadcast(rb[:, :], zr[0:1, 0:QW])

                        # yT = (expS^T-weighted V) * (1/Z), V = x natural
                        yts = []
                        for e in range(ET):
                            ps = ps_main.tile([128, 512], F32, tag="ps")
                            for ki in range(kis):
                                nc.tensor.matmul(
                                    ps[:, 0:QW],
                                    _mm(xn[ki][:, 128 * e : 128 * (e + 1)]),
                                    _mm(exps[ki][:, :]),
                                    start=(ki == 0),
                                    stop=(ki == kis - 1),
                                )
                            yt = p_yt.tile([128, QW], F32, tag="yt")
                            nc.vector.tensor_mul(yt[:, :], ps[:, 0:QW], rb[:, :])
                            yts.append(yt)

                        # ln(y)
                        ylns = [p_yln.tile([128, QW], MMF, tag="yln", name=f"yln{_}") for _ in range(ET)]
                        ln_partition([y[:, :] for y in yts], [y[:, :] for y in ylns], QW)

                        # encoder_v + xy (in place into xq)
                        for g in range(8):
                            slab = p_wslab.tile([128, ET, 256], MMF, tag="wslab")
                            nc.sync.dma_start(
                                slab[:, :, :],
                                encv_d[h]
                                .ap()
                                .rearrange("(c p) n -> p c n", p=128)[
                                    :, :, 256 * g : 256 * (g + 1)
                                ],
                            )
                            for j in range(2):
                                ncx = 2 * g + j
                                ps = ps_main.tile([128, 512], F32, tag="ps")
                                for e in range(ET):
                                    nc.tensor.matmul(
                                        ps[:, 0:QW],
                                        _mm(slab[:, e, 128 * j : 128 * (j + 1)]),
                                        _mm(ylns[e][:, :]),
                                        start=(e == 0),
                                        stop=(e == ET - 1),
                                    )
                                ys = p_stage.tile([128, QW], F32, tag="ys")
                                nc.scalar.activation(ys[:, :], ps[:, 0:QW], AF.Relu)
                                nc.vector.tensor_mul(
                                    xq[:, ncx, :], ys[:, :], xq[:, ncx, :]
                                )

                        # decoder: ymlp[e, qs:qs+QW] (+)= sum_n dec[n,e]^T xy[n,q]
                        for e in range(ET):
                            slab = p_wslab.tile([128, NT, 128], MMF, tag="wslab")
                            nc.sync.dma_start(
                                slab[:, :, :],
                                dec_d[h]
                                .ap()
                                .rearrange("(n p) c -> p n c", p=128)[
                                    :, :, 128 * e : 128 * (e + 1)
                                ],
                            )
                            ps = ps_main.tile([128, 512], F32, tag="ps")
                            for ncx in range(NT):
                                nc.tensor.matmul(
                                    ps[:, 0:QW],
                                    _mm(slab[:, ncx, :]),
                                    _mm(xq[:, ncx, :]),
                                    start=(ncx == 0),
                                    stop=(ncx == NT - 1),
                                )
                            if h == 0:
                                nc.scalar.copy(ymlp[e][:, qs : qs + QW], ps[:, 0:QW])
                            else:
                                dst = ymlp[e][:, bass.ds(qs, QW)]
                                nc.vector.tensor_add(dst, dst, ps[:, 0:QW])

                # ---- Phase C: AllReduce + LN chain + transpose ----
                for e in range(ET):
                    nc.sync.dma_start(
                        cc_in[128 * e : 128 * (e + 1), :], ymlp[e][:, :]
                    )
                if use_cc:
                    nc.gpsimd.collective_compute(
                        "AllReduce",
                        ALU.add,
                        replica_groups=[list(range(NCORES))],
                        ins=[cc_in.ap().opt()],
                        outs=[cc_out.ap().opt()],
                    )
                else:
                    nc.sync.dma_start(cc_out[:, :], cc_in[:, :])
                ym = ymlp
                for e in range(ET):
                    nc.sync.dma_start(ym[e][:, :], cc_out[128 * e : 128 * (e + 1), :])
                for half in range(2):
                    sl = slice(512 * half, 512 * (half + 1))
                    # ln(y_mlp)
                    ln_partition(
                        [ym[e][:, sl] for e in range(ET)],
                        [ym[e][:, sl] for e in range(ET)],
                        512,
                    )
                    # x + ln(y_mlp)
                    for e in range(ET):
                        nc.vector.tensor_add(ym[e][:, sl], ym[e][:, sl], xT[e][:, sl])
                    # ln(...) -> new xT
                    ln_partition(
                        [ym[e][:, sl] for e in range(ET)],
                        [xT[e][:, sl] for e in range(ET)],
                        512,
                    )
                # transpose xT -> x natural
                for t in range(TT):
                    for e in range(ET):
                        transpose128(
                            xn[t][:, 128 * e : 128 * (e + 1)],
                            xT[e][:, 128 * t : 128 * (t + 1)],
                        )

            # ---------------- lm_head ----------------
            if with_lm:
                nvt = (VS + 255) // 256  # 16 v-tiles of <=256
                for vt in range(nvt):
                    w = min(256, VS - 256 * vt)
                    slab = p_wslab.tile([128, ET, 256], MMF, tag="wslab")
                    nc.sync.dma_start(
                        slab[:, :, 0:w],
                        lmT_d.ap().rearrange("(c p) v -> p c v", p=128)[
                            :, :, 256 * vt : 256 * vt + w
                        ],
                    )
                    for t in range(TT):
                        ps = ps_main.tile([128, 512], F32, tag="ps")
                        for e in range(ET):
                            nc.tensor.matmul(
                                ps[:, 0:w],
                                _mm(xT[e][:, 128 * t : 128 * (t + 1)]),
                                _mm(slab[:, e, 0:w]),
                                start=(e == 0),
                                stop=(e == ET - 1),
                            )
                        ot = p_stage.tile([128, 256], F32, tag="lmout")
                        nc.scalar.copy(ot[:, 0:w], ps[:, 0:w])
                        nc.sync.dma_start(
                            out_d[128 * t : 128 * (t + 1), 256 * vt : 256 * vt + w],
                            ot[:, 0:w],
                        )

    nc.compile()
    return nc


_CACHED_NC = None


def _get_nc():
    global _CACHED_NC
    if _CACHED_NC is None:
        _CACHED_NC = build_program()
    return _CACHED_NC


def make_in_maps(idx, embed, encoder, encoder_v, decoder, lm_head, ln_w, ln_b):
    idx = np.asarray(idx)
    embed = np.asarray(embed, dtype=np.float32)
    encoder = np.asarray(encoder, dtype=np.float32)
    encoder_v = np.asarray(encoder_v, dtype=np.float32)
    decoder = np.asarray(decoder, dtype=np.float32)
    lm_head = np.asarray(lm_head, dtype=np.float32)
    ln_w = np.asarray(ln_w, dtype=np.float32)
    ln_b = np.asarray(ln_b, dtype=np.float32)
    assert idx.shape == (B, T)

    xe = embed[idx.astype(np.int64).reshape(-1)]  # [T, E] host gather
    dec3 = decoder.reshape(NH, N, E)

    in_maps = []
    for c in range(NCORES):
        p = c // 2
        par = c % 2
        hF = 3 * p + par
        hS = 3 * p + 2
        in_maps.append(
            {
                "xe": np.ascontiguousarray(xe),
                "encF": np.ascontiguousarray(encoder[hF]),
                "encS": np.ascontiguousarray(encoder[hS]),
                "encvF": np.ascontiguousarray(encoder_v[hF]),
                "encvS": np.ascontiguousarray(encoder_v[hS]),
                "decF": np.ascontiguousarray(dec3[hF]),
                "decS": np.ascontiguousarray(dec3[hS]),
                "lmT": np.ascontiguousarray(lm_head[VS * c : VS * (c + 1), :].T),
                "lnw": np.ascontiguousarray(ln_w),
                "lnb": np.ascontiguousarray(ln_b),
                "maskS": _host_masks_S(par),
            }
        )
    return in_maps


def kernel(idx, embed, encoder, encoder_v, decoder, lm_head, ln_w, ln_b, n_layer):
    assert int(n_layer) == NL
    in_maps = make_in_maps(
        idx, embed, encoder, encoder_v, decoder, lm_head, ln_w, ln_b
    )
    nc = _get_nc()
    res = run_bass_kernel_spmd(nc, in_maps, core_ids=list(range(NCORES)))
    logits = np.concatenate(
        [res.results[c]["out"] for c in range(NCORES)], axis=1
    )  # [T, VOCAB]
    return logits.reshape(B, T, VOCAB).astype(np.float32)


if __name__ == "__main__":
    nc = build_program(n_layer=1, with_lm=False)
    print("built ok")
